# revision 1
# baseline (speedup 1.0000x reference)
"""GAT encoder (gnn_message_passing) on 8 trn2 NeuronCores via Bass.

Strategy (graph-parallel, dst-sharded):
  Launch 1 (sharded by node range): h = x@W1, es = x@(W1@att_src),
    ed = x@(W1@att_dst). Each core writes haug rows [h fp32 | es fp32]
    (129 floats = 516B) for its 6250 nodes, plus ed.
  Host: concatenate haug shards (+1 dummy row with es=-1e30), permute ed
    into degree-sorted window layout, route edges to dst-owner cores.
  Launch 2 (per core, dst windows of 128 degree-sorted nodes): indirect-DMA
    gather of haug rows for all edge slots (padded to per-window uniform
    chunk count); alpha = sigmoid(es_src + ed_dst); ex = exp(alpha)
    (max-subtraction dropped: alpha in (0,1) so exp is stable; softmax
    weights are mathematically identical); per-chunk scale rows by ex and
    accumulate via identity-stationary matmuls into PSUM; denominator =
    free-dim reduce of ex minus pad count; out = ELU(acc/den) @ W2.
"""
import os
import sys
import time

sys.path.insert(0, "/opt/trn_rl_repo")

import numpy as np

N, E = 50000, 800000
IN, HID, OUT = 256, 128, 128
NCORES = 8
NPC = N // NCORES            # nodes per core
NW = NPC // 128              # windows per core (49 when NPC=6272... 6250/128)
assert NPC % 2 == 0
NW = (NPC + 127) // 128      # 49 windows; last window partial (6250 = 48*128+106)
ROWF = HID + 1               # floats per haug row (h + es)
DUMMY = N                    # dummy haug row index (es = -1e30)
GCOLS = 32                   # max idx columns per gather call

_timings = {}


def _patch_env():
    """Tile/perfetto compatibility patches for this container."""
    import concourse.tile as tile
    from concourse.tile import ScopedClock

    def _drain_and_barrier_split(self, tick_clock, wait_clock):
        nc = self.nc
        probe = nc.sync.nop()
        wait_clock.add_sem_waits(
            probe.ins, ScopedClock({None: tick_clock.global_clock})
        )
        waits = list(probe.ins.sync_info.on_wait or [])
        probe.ins.sync_info.on_wait = []
        from concourse import mybir

        for w in waits:
            inst = nc.sync.nop()
            if inst.ins.sync_info is None:
                inst.ins.sync_info = mybir.SyncInfo(on_wait=[w], on_update=[])
            else:
                inst.ins.sync_info.on_wait = [w]
        nc.sync.drain()
        nc.all_engine_barrier()
        assert self.sems is not None
        popped = nc._tile_sem_poison_stack.pop()
        assert popped is self._sem_poison
        nc.clear_and_free_semaphores(list(self.sems.allocated().values()))
        nc.all_engine_barrier()

    tile.TileContext._drain_and_barrier = _drain_and_barrier_split


_patch_env()


def _patch_perfetto():
    try:
        from gauge import trn_perfetto

        cls = trn_perfetto.TrnPerfettoConv
        if not getattr(cls, "_no_hlo_patched", False):
            _orig_init = cls.__init__

            def _init_no_hlo(self, *a, **k):
                k["annotate_hlo"] = False
                if len(a) >= 2:
                    a = (a[0], False) + a[2:]
                _orig_init(self, *a, **k)

            cls.__init__ = _init_no_hlo
            cls._no_hlo_patched = True
    except Exception:
        pass


import concourse.bass as bass
import concourse.bacc as bacc
import concourse.tile as tile
from concourse import mybir
from concourse.bass_utils import run_bass_kernel_spmd
from concourse.masks import make_identity

F32 = mybir.dt.float32
I32 = mybir.dt.int32
AF = mybir.ActivationFunctionType
ALU = mybir.AluOpType


# ---------------------------------------------------------------- phase 1
def build_phase1():
    nc = bacc.Bacc("TRN2", target_bir_lowering=True)
    ntiles = (NPC + 127) // 128
    npad = ntiles * 128
    xT = nc.dram_tensor("xT", [IN, npad], F32, kind="ExternalInput")
    w1 = nc.dram_tensor("w1", [IN, HID], F32, kind="ExternalInput")
    w1a = nc.dram_tensor("w1a", [IN, 1], F32, kind="ExternalInput")
    w1d = nc.dram_tensor("w1d", [IN, 1], F32, kind="ExternalInput")
    haug = nc.dram_tensor("haug", [npad, ROWF], F32, kind="ExternalOutput")
    edo = nc.dram_tensor("edo", [128, ntiles], F32, kind="ExternalOutput")

    with tile.TileContext(nc) as tc:
        with (
            tc.tile_pool(name="sbuf", bufs=3) as pool,
            tc.tile_pool(name="cpool", bufs=1) as cpool,
            tc.tile_pool(name="psum", bufs=2, space="PSUM") as psum,
        ):
            w1_t = cpool.tile([128, IN // 128, HID], F32)
            nc.sync.dma_start(
                out=w1_t[:], in_=w1[:].rearrange("(a k) f -> k a f", k=128)
            )
            w1a_t = cpool.tile([128, IN // 128, 1], F32)
            nc.sync.dma_start(
                out=w1a_t[:], in_=w1a[:].rearrange("(a k) f -> k a f", k=128)
            )
            w1d_t = cpool.tile([128, IN // 128, 1], F32)
            nc.sync.dma_start(
                out=w1d_t[:], in_=w1d[:].rearrange("(a k) f -> k a f", k=128)
            )
            ed_sb = cpool.tile([128, ntiles], F32)

            for t in range(ntiles):
                xt = pool.tile([128, IN // 128, 128], F32, tag="xt")
                nc.sync.dma_start(
                    out=xt[:],
                    in_=xT[:, t * 128 : (t + 1) * 128].rearrange(
                        "(a k) n -> k a n", k=128
                    ),
                )
                hp = psum.tile([128, HID], F32, tag="hp")
                esp = psum.tile([128, 1], F32, tag="esp")
                edp = psum.tile([128, 1], F32, tag="edp")
                for a in range(IN // 128):
                    st = a == 0
                    sp = a == IN // 128 - 1
                    nc.tensor.matmul(
                        out=hp[:], lhsT=xt[:, a], rhs=w1_t[:, a], start=st, stop=sp
                    )
                    nc.tensor.matmul(
                        out=esp[:], lhsT=xt[:, a], rhs=w1a_t[:, a], start=st, stop=sp
                    )
                    nc.tensor.matmul(
                        out=edp[:], lhsT=xt[:, a], rhs=w1d_t[:, a], start=st, stop=sp
                    )
                ha = pool.tile([128, ROWF], F32, tag="ha")
                nc.scalar.activation(ha[:, 0:HID], hp[:], AF.Copy)
                nc.vector.tensor_copy(ha[:, HID : HID + 1], esp[:])
                nc.vector.tensor_copy(ed_sb[:, t : t + 1], edp[:])
                nc.sync.dma_start(
                    out=haug[t * 128 : (t + 1) * 128, :], in_=ha[:]
                )
            nc.sync.dma_start(out=edo[:], in_=ed_sb[:])
    nc.finalize()
    return nc


# ---------------------------------------------------------------- phase 2
def build_phase2(nchunks, groups):
    """nchunks: per-window chunk counts (uniform across cores).
    groups: list of (w_start, w_end) gather groups."""
    TOT = int(np.sum(nchunks))
    offs = np.zeros(len(nchunks) + 1, dtype=int)
    offs[1:] = np.cumsum(nchunks)

    nc = bacc.Bacc("TRN2", target_bir_lowering=True)
    haug = nc.dram_tensor("haug", [N + 1, ROWF], F32, kind="ExternalInput")
    idxs = nc.dram_tensor("idxs", [128, TOT], I32, kind="ExternalInput")
    edw = nc.dram_tensor("edw", [128, NW], F32, kind="ExternalInput")
    pcw = nc.dram_tensor("pcw", [128, NW], F32, kind="ExternalInput")
    w2 = nc.dram_tensor("w2", [HID, OUT], F32, kind="ExternalInput")
    y = nc.dram_tensor("y", [NW * 128, OUT], F32, kind="ExternalOutput")

    with tile.TileContext(nc) as tc:
        with (
            tc.tile_pool(name="gpool", bufs=3) as gpool,
            tc.tile_pool(name="spool", bufs=6) as spool,
            tc.tile_pool(name="cpool", bufs=1) as cpool,
            tc.tile_pool(name="psum", bufs=2, space="PSUM") as psum,
            tc.tile_pool(name="psum2", bufs=2, space="PSUM") as psum2,
        ):
            ident = cpool.tile([128, 128], F32)
            make_identity(nc, ident[:])
            w2_t = cpool.tile([HID, OUT], F32)
            nc.sync.dma_start(out=w2_t[:], in_=w2[:])
            edw_t = cpool.tile([128, NW], F32)
            nc.sync.dma_start(out=edw_t[:], in_=edw[:])
            pcw_t = cpool.tile([128, NW], F32)
            nc.sync.dma_start(out=pcw_t[:], in_=pcw[:])

            for (w0, w1_) in groups:
                c0, c1 = int(offs[w0]), int(offs[w1_])
                ncols = c1 - c0
                it = gpool.tile([128, ncols], I32, tag="it")
                nc.sync.dma_start(out=it[:], in_=idxs[:, c0:c1])
                gt = gpool.tile([128, ncols * ROWF], F32, tag="gt")
                # HW dynamic-offset DGE applies ONE offset per partition per
                # call (scalar_dynamic_offset level), so issue one indirect
                # DMA per idx column (128 rows per call).
                for cc in range(ncols):
                    nc.gpsimd.indirect_dma_start(
                        out=gt[:, cc * ROWF : (cc + 1) * ROWF],
                        out_offset=None,
                        in_=haug[:],
                        in_offset=bass.IndirectOffsetOnAxis(
                            ap=it[:, cc : cc + 1], axis=0
                        ),
                    )
                gt3 = gt[:].rearrange("p (c f) -> p c f", f=ROWF)
                for w in range(w0, w1_):
                    nch = int(nchunks[w])
                    lo = int(offs[w]) - c0
                    # alpha = sigmoid(es + ed); ex = exp(alpha)
                    alpha = spool.tile([128, nch], F32, tag="alpha")
                    nc.scalar.activation(
                        alpha[:],
                        gt3[:, lo : lo + nch, HID : HID + 1].rearrange(
                            "p c f -> p (c f)"
                        ),
                        AF.Sigmoid,
                        bias=edw_t[:, w : w + 1],
                    )
                    ex = spool.tile([128, nch], F32, tag="ex")
                    nc.scalar.activation(ex[:], alpha[:], AF.Exp)
                    # denominator
                    den = spool.tile([128, 1], F32, tag="den")
                    nc.vector.reduce_sum(
                        den[:], ex[:], axis=mybir.AxisListType.X
                    )
                    nc.vector.tensor_tensor(
                        out=den[:], in0=den[:], in1=pcw_t[:, w : w + 1],
                        op=ALU.subtract,
                    )
                    nc.vector.tensor_scalar_max(den[:], den[:], 0.5)
                    recip = spool.tile([128, 1], F32, tag="recip")
                    nc.vector.reciprocal(recip[:], den[:])
                    # scale all chunks by ex (broadcast along feature dim)
                    gs = spool.tile([128, nch * HID], F32, tag="gs")
                    nc.vector.tensor_tensor(
                        out=gs[:].rearrange("p (c f) -> p c f", f=HID),
                        in0=gt3[:, lo : lo + nch, 0:HID],
                        in1=ex[:, :, None].to_broadcast([128, nch, HID]),
                        op=ALU.mult,
                    )
                    acc = psum.tile([128, HID], F32, tag="acc")
                    for c in range(nch):
                        nc.tensor.matmul(
                            out=acc[:],
                            lhsT=ident[:],
                            rhs=gs[:, c * HID : (c + 1) * HID],
                            start=(c == 0),
                            stop=(c == nch - 1),
                        )
                    # ELU(acc * recip): x - relu(x) = min(x,0)
                    xs = spool.tile([128, HID], F32, tag="xs")
                    nc.vector.tensor_scalar(
                        out=xs[:], in0=acc[:], scalar1=recip[:],
                        scalar2=None, op0=ALU.mult,
                    )
                    mm = spool.tile([128, HID], F32, tag="mm")
                    nc.vector.tensor_scalar_min(mm[:], xs[:], 0.0)
                    ee = spool.tile([128, HID], F32, tag="ee")
                    nc.scalar.activation(ee[:], mm[:], AF.Exp)
                    rr = spool.tile([128, HID], F32, tag="rr")
                    nc.vector.tensor_scalar(
                        out=rr[:], in0=xs[:], scalar1=0.0, scalar2=-1.0,
                        op0=ALU.max, op1=ALU.add,
                    )
                    h1 = spool.tile([128, HID], F32, tag="h1")
                    nc.vector.tensor_tensor(
                        out=h1[:], in0=rr[:], in1=ee[:], op=ALU.add
                    )
                    # y_w = h1 @ W2  (transpose h1 on PE, then matmul)
                    h1tp = psum2.tile([128, HID], F32, tag="h1tp")
                    nc.tensor.transpose(
                        out=h1tp[:], in_=h1[:], identity=ident[:]
                    )
                    h1t = spool.tile([128, HID], F32, tag="h1t")
                    nc.scalar.activation(h1t[:], h1tp[:], AF.Copy)
                    yp = psum2.tile([128, OUT], F32, tag="yp")
                    nc.tensor.matmul(
                        out=yp[:], lhsT=h1t[:], rhs=w2_t[:],
                        start=True, stop=True,
                    )
                    yt = spool.tile([128, OUT], F32, tag="yt")
                    nc.scalar.activation(yt[:], yp[:], AF.Copy)
                    nc.sync.dma_start(
                        out=y[w * 128 : (w + 1) * 128, :], in_=yt[:]
                    )
    nc.finalize()
    return nc


# ---------------------------------------------------------------- host glue
def kernel(x, edge_index, W1, att_src, att_dst, W2):
    x = np.asarray(x, dtype=np.float32)
    edge_index = np.asarray(edge_index)
    W1 = np.asarray(W1, dtype=np.float32)
    att_src = np.asarray(att_src, dtype=np.float32)
    att_dst = np.asarray(att_dst, dtype=np.float32)
    W2 = np.asarray(W2, dtype=np.float32)

    src = edge_index[0].astype(np.int64)
    dst = edge_index[1].astype(np.int64)

    # ---- phase 1: sharded h/es/ed compute
    xT = np.ascontiguousarray(x.T)  # [IN, N]
    w1a = (W1 @ att_src).reshape(IN, 1).astype(np.float32)
    w1d = (W1 @ att_dst).reshape(IN, 1).astype(np.float32)
    ntiles = (NPC + 127) // 128
    npad = ntiles * 128

    nc1 = build_phase1()
    in_maps1 = []
    for c in range(NCORES):
        sh = xT[:, c * NPC : (c + 1) * NPC]
        if sh.shape[1] < npad:
            sh = np.concatenate(
                [sh, np.zeros((IN, npad - sh.shape[1]), np.float32)], axis=1
            )
        in_maps1.append(
            {"xT": np.ascontiguousarray(sh), "w1": W1, "w1a": w1a, "w1d": w1d}
        )
    trace = os.environ.get("BASS_GAT_TRACE") == "1"
    tkw = dict(trace=True, trace_cores=[0]) if trace else {}
    if trace:
        _patch_perfetto()
    t0 = time.time()
    res1 = run_bass_kernel_spmd(nc1, in_maps1, core_ids=list(range(NCORES)), **tkw)
    _timings["phase1_wall"] = time.time() - t0
    _timings["phase1_ns"] = res1.exec_time_ns

    haug_full = np.zeros((N + 1, ROWF), np.float32)
    ed_full = np.zeros(N, np.float32)
    for c in range(NCORES):
        haug_full[c * NPC : (c + 1) * NPC] = res1.results[c]["haug"][:NPC]
        ed_full[c * NPC : (c + 1) * NPC] = (
            res1.results[c]["edo"].T.ravel()[:NPC]
        )
    haug_full[N, HID] = -1e30  # dummy row: es=-inf, h=0

    # ---- host edge routing: per-core degree-sorted windows
    deg = np.bincount(dst, minlength=N)
    orders = []
    nch_per_core = np.zeros((NCORES, NW), np.int64)
    for c in range(NCORES):
        dl = deg[c * NPC : (c + 1) * NPC]
        order = np.argsort(-dl, kind="stable")
        orders.append(order)
        dls = dl[order]
        for w in range(NW):
            j0 = w * 128
            nch_per_core[c, w] = dls[j0] if j0 < NPC else 0
    nchunks = np.maximum(nch_per_core.max(axis=0), 1)
    offs = np.zeros(NW + 1, dtype=np.int64)
    offs[1:] = np.cumsum(nchunks)
    TOT = int(offs[-1])

    # gather groups
    groups = []
    w0 = 0
    while w0 < NW:
        w1_ = w0 + 1
        while w1_ < NW and offs[w1_ + 1] - offs[w0] <= GCOLS:
            w1_ += 1
        groups.append((w0, w1_))
        w0 = w1_

    # per-core idx/padcnt/ed arrays
    eorder = np.argsort(dst, kind="stable")
    src_s = src[eorder]
    estarts = np.zeros(N + 1, np.int64)
    estarts[1:] = np.cumsum(deg)

    in_maps2 = []
    for c in range(NCORES):
        order = orders[c]
        rank = np.empty(NPC, np.int64)
        rank[order] = np.arange(NPC)
        idx_arr = np.full((128, TOT), DUMMY, np.int32)
        padcnt = np.zeros((128, NW), np.float32)
        edw = np.zeros((128, NW), np.float32)
        for wloc in range(NW):
            j0 = wloc * 128
            nodes = order[j0 : j0 + 128]  # local ids, len<=128
            for p, j in enumerate(nodes):
                g = c * NPC + j
                d = deg[g]
                s0 = estarts[g]
                cols = slice(int(offs[wloc]), int(offs[wloc]) + int(d))
                idx_arr[p, cols] = src_s[s0 : s0 + d]
                padcnt[p, wloc] = nchunks[wloc] - d
                edw[p, wloc] = ed_full[g]
            for p in range(len(nodes), 128):
                padcnt[p, wloc] = nchunks[wloc]
        in_maps2.append(
            {
                "haug": haug_full,
                "idxs": idx_arr,
                "edw": edw,
                "pcw": padcnt,
                "w2": W2,
            }
        )

    nc2 = build_phase2(nchunks, groups)
    t0 = time.time()
    res2 = run_bass_kernel_spmd(nc2, in_maps2, core_ids=list(range(NCORES)), **tkw)
    _timings["phase2_wall"] = time.time() - t0
    _timings["phase2_ns"] = res2.exec_time_ns

    out = np.zeros((N, OUT), np.float32)
    for c in range(NCORES):
        yv = res2.results[c]["y"]
        order = orders[c]
        valid = min(NPC, NW * 128)
        out[c * NPC + order[:valid]] = yv[:valid]
    return out



# revision 7
# speedup vs baseline: 1.1727x; 1.1727x over previous
"""GAT encoder (gnn_message_passing) on 8 trn2 NeuronCores via Bass.

Strategy (graph-parallel, dst-sharded), v2:
  Phase 1 (node-sharded): one fp16 matmul chain per 128-node tile against
    [W1 | W1@att_src | W1@att_dst] -> rows [h(128) | es | ed] fp16.
  Host: build gather table htab (h rows, 256B fp16), route edges to
    dst-owner cores into windows of 128 dst nodes sorted by (low-degree,
    high-degree); per-slot es+ed bias; per-window pad counts.
  Phase 2 (per core): dma_gather bulk-fetches edge-slot h rows.  int16
    indices limit a gather table to 32768 rows, so each window has
    nchA low-src columns + nchB high-src columns, fetched by two calls
    per group from the two table halves (pad slots read a zero dummy row;
    every index is valid - HW emits descriptors for the first
    num_idxs_reg entries and skips nothing).  Attention without act-table
    swaps: exp(sigmoid(z)) = e^0.5 * exp(tanh(z/2)/2); the constant
    cancels in the softmax (pad slots use es=-60000 -> e^-0.5, removed
    from the denominator via pcw).  ex applied via one DVE fp16 multiply
    per pass per group; per-window PSUM accumulation via fp16 identity
    matmuls; ELU via max(x,0)+exp(min(x,0))-1 with the -1 folded into
    the output bias; y produced transposed and fixed up on host.
"""
import os
import sys
import time

sys.path.insert(0, "/opt/trn_rl_repo")

import numpy as np

N, E = 50000, 800000
IN, HID, OUT = 256, 128, 128
NCORES = 8
NPC = N // NCORES            # nodes per core (6250)
NW = (NPC + 127) // 128      # 49 windows; last partial (6250 = 48*128+106)
NPAD = NW * 128              # 6272
GCOLS = 64                   # max total gather columns per group
CALLCOLS = int(os.environ.get("BASS_GAT_CALLCOLS", "64"))
SINGLE_PACKET = os.environ.get("BASS_GAT_SP", "0") == "1"

TABROWS = N + 3              # row 0 = zero (low dummy), 1..N = node src+1,
LOWROWS = 32768              # row N+2 = zero (high dummy)
HIBASE = 32767
DUMMY_HI_LOCAL = N + 2 - HIBASE

_timings = {}


def _patch_env():
    """Tile/perfetto compatibility patches for this container."""
    import concourse.tile as tile
    from concourse.tile import ScopedClock

    def _drain_and_barrier_split(self, tick_clock, wait_clock):
        nc = self.nc
        probe = nc.sync.nop()
        wait_clock.add_sem_waits(
            probe.ins, ScopedClock({None: tick_clock.global_clock})
        )
        waits = list(probe.ins.sync_info.on_wait or [])
        probe.ins.sync_info.on_wait = []
        from concourse import mybir

        for w in waits:
            inst = nc.sync.nop()
            if inst.ins.sync_info is None:
                inst.ins.sync_info = mybir.SyncInfo(on_wait=[w], on_update=[])
            else:
                inst.ins.sync_info.on_wait = [w]
        nc.sync.drain()
        nc.all_engine_barrier()
        assert self.sems is not None
        popped = nc._tile_sem_poison_stack.pop()
        assert popped is self._sem_poison
        nc.clear_and_free_semaphores(list(self.sems.allocated().values()))
        nc.all_engine_barrier()

    tile.TileContext._drain_and_barrier = _drain_and_barrier_split


_patch_env()


def _patch_perfetto():
    try:
        from gauge import trn_perfetto

        cls = trn_perfetto.TrnPerfettoConv
        if not getattr(cls, "_no_hlo_patched", False):
            _orig_init = cls.__init__

            def _init_no_hlo(self, *a, **k):
                k["annotate_hlo"] = False
                if len(a) >= 2:
                    a = (a[0], False) + a[2:]
                _orig_init(self, *a, **k)

            cls.__init__ = _init_no_hlo
            cls._no_hlo_patched = True
    except Exception:
        pass


import concourse.bass as bass
import concourse.bacc as bacc
import concourse.tile as tile
from concourse import mybir
from concourse.bass_utils import run_bass_kernel_spmd
from concourse.masks import make_identity

F32 = mybir.dt.float32
F16 = mybir.dt.float16
I16 = mybir.dt.int16
AF = mybir.ActivationFunctionType
ALU = mybir.AluOpType


# ---------------------------------------------------------------- phase 1
def build_phase1():
    """h/es/ed for this core's nodes: one fp16 matmul chain per tile."""
    nc = bacc.Bacc("TRN2", target_bir_lowering=True)
    xT = nc.dram_tensor("xT", [IN, NPAD], F16, kind="ExternalInput")
    wcat = nc.dram_tensor("wcat", [IN, HID + 2], F16, kind="ExternalInput")
    haug = nc.dram_tensor("haug", [NPAD, HID + 2], F16, kind="ExternalOutput")

    RF = HID + 2  # 130
    TB = 4        # tiles per DMA batch

    with tile.TileContext(nc) as tc:
        with (
            tc.tile_pool(name="sbuf", bufs=3) as pool,
            tc.tile_pool(name="cpool", bufs=1) as cpool,
            tc.tile_pool(name="psum", bufs=4, space="PSUM") as psum,
        ):
            w_t = cpool.tile([128, IN // 128, RF], F16)
            nc.sync.dma_start(
                out=w_t[:], in_=wcat[:].rearrange("(a k) f -> k a f", k=128)
            )
            nb = (NW + TB - 1) // TB
            for b in range(nb):
                t0 = b * TB
                tn = min(TB, NW - t0)
                xt = pool.tile([128, IN // 128, TB * 128], F16, tag="xt")
                nc.sync.dma_start(
                    out=xt[:, :, : tn * 128],
                    in_=xT[:, t0 * 128 : (t0 + tn) * 128].rearrange(
                        "(a k) n -> k a n", k=128
                    ),
                )
                ha = pool.tile([128, TB, RF], F16, tag="ha")
                for t in range(tn):
                    hp = psum.tile([128, RF], F32, tag="hp")
                    for a in range(IN // 128):
                        nc.tensor.matmul(
                            out=hp[:],
                            lhsT=xt[:, a, t * 128 : (t + 1) * 128],
                            rhs=w_t[:, a],
                            start=(a == 0),
                            stop=(a == IN // 128 - 1),
                        )
                    nc.scalar.activation(ha[:, t], hp[:], AF.Copy)
                nc.sync.dma_start(
                    out=haug[t0 * 128 : (t0 + tn) * 128, :].rearrange(
                        "(t k) f -> k t f", k=128
                    ),
                    in_=ha[:, :tn],
                )
    nc.finalize()
    return nc


# ---------------------------------------------------------------- phase 2
def build_phase2(nchA, nchB, groups):
    """Per-window low/high chunk counts and group spans."""
    NWl = len(nchA)
    offsA = np.zeros(NWl + 1, dtype=int)
    offsA[1:] = np.cumsum(nchA)
    offsB = np.zeros(NWl + 1, dtype=int)
    offsB[1:] = np.cumsum(nchB)
    TOTA, TOTB = int(offsA[-1]), int(offsB[-1])
    # idx segment start per group (8 wrapped cols per gather col, A then B)
    ico = np.zeros(len(groups) + 1, dtype=int)
    for g, (w0, w1_) in enumerate(groups):
        na = int(offsA[w1_] - offsA[w0])
        nb_ = int(offsB[w1_] - offsB[w0])
        ico[g + 1] = ico[g] + 8 * (na + nb_)
    ITOT = int(ico[-1])

    nc = bacc.Bacc("TRN2", target_bir_lowering=True)
    htab = nc.dram_tensor("htab", [TABROWS, HID], F16, kind="ExternalInput")
    idxs = nc.dram_tensor("idxs", [128, ITOT], I16, kind="ExternalInput")
    esw = nc.dram_tensor("esw", [128, TOTA + TOTB], F16, kind="ExternalInput")
    pcw = nc.dram_tensor("pcw", [128, NW], F32, kind="ExternalInput")
    w2 = nc.dram_tensor("w2", [HID, OUT], F16, kind="ExternalInput")
    c2n = nc.dram_tensor("c2n", [OUT, 1], F32, kind="ExternalInput")
    yT = nc.dram_tensor("yT", [OUT, NW * 128], F16, kind="ExternalOutput")

    with tile.TileContext(nc) as tc:
        with (
            tc.tile_pool(name="gpool", bufs=3) as gpool,
            tc.tile_pool(name="mpool", bufs=2) as mpool,
            tc.tile_pool(name="spool", bufs=6) as spool,
            tc.tile_pool(name="hpool", bufs=3) as hpool,
            tc.tile_pool(name="cpool", bufs=1) as cpool,
            tc.tile_pool(name="psum", bufs=4, space="PSUM") as psum,
            tc.tile_pool(name="psumt", bufs=2, space="PSUM") as psumt,
            tc.tile_pool(name="psumy", bufs=2, space="PSUM") as psumy,
        ):
            ident = cpool.tile([128, 128], F16)
            make_identity(nc, ident[:])
            w2_t = cpool.tile([HID, OUT], F16)
            nc.sync.dma_start(out=w2_t[:], in_=w2[:])
            c2n_t = cpool.tile([OUT, 1], F32)
            nc.sync.dma_start(out=c2n_t[:], in_=c2n[:])
            pcw_t = cpool.tile([128, NW], F32)
            nc.sync.dma_start(out=pcw_t[:], in_=pcw[:])
            it_t = cpool.tile([128, ITOT], I16)
            nc.sync.dma_start(out=it_t[:], in_=idxs[:])
            esw_t = cpool.tile([128, TOTA + TOTB], F16)
            nc.sync.dma_start(out=esw_t[:], in_=esw[:])

            for g, (w0, w1_) in enumerate(groups):
                a0, a1 = int(offsA[w0]), int(offsA[w1_])
                b0, b1 = int(offsB[w0]), int(offsB[w1_])
                na, nb_ = a1 - a0, b1 - b0
                i0 = int(ico[g])

                gtA = gpool.tile([128, na * HID], F16, tag="gtA")
                gA3 = gtA[:].rearrange("p (c f) -> p c f", f=HID)
                for s0 in range(0, na, CALLCOLS):
                    sn = min(CALLCOLS, na - s0)
                    nc.gpsimd.dma_gather(
                        out_ap=gA3[:, s0 : s0 + sn],
                        in_ap=htab[:LOWROWS],
                        idxs_ap=it_t[:, i0 + 8 * s0 : i0 + 8 * (s0 + sn)],
                        num_idxs=128 * sn,
                        num_idxs_reg=128 * sn,
                        elem_size=HID,
                        single_packet=SINGLE_PACKET,
                    )
                gtB = gpool.tile([128, max(nb_, 1) * HID], F16, tag="gtB")
                gB3 = gtB[:].rearrange("p (c f) -> p c f", f=HID)
                for s0 in range(0, nb_, CALLCOLS):
                    sn = min(CALLCOLS, nb_ - s0)
                    nc.gpsimd.dma_gather(
                        out_ap=gB3[:, s0 : s0 + sn],
                        in_ap=htab[HIBASE:],
                        idxs_ap=it_t[
                            :, i0 + 8 * (na + s0) : i0 + 8 * (na + s0 + sn)
                        ],
                        num_idxs=128 * sn,
                        num_idxs_reg=128 * sn,
                        elem_size=HID,
                        single_packet=SINGLE_PACKET,
                    )
                # t = tanh(z/2); ex = exp(t/2)  (softmax scale e^0.5 cancels)
                twA = spool.tile([128, na], F32, tag="twA")
                nc.scalar.activation(
                    twA[:], esw_t[:, a0:a1], AF.Tanh, scale=0.5
                )
                ex2A = spool.tile([128, na, 2], F16, tag="ex2A")
                nc.scalar.activation(
                    ex2A[:],
                    twA[:, :, None].to_broadcast([128, na, 2]),
                    AF.Exp,
                    scale=0.5,
                )
                gsA = mpool.tile([128, na * HID], F16, tag="gsA")
                nc.vector.tensor_tensor(
                    out=gsA[:].rearrange("p (c a two) -> p c a two", a=64, two=2),
                    in0=gA3.rearrange("p c (a two) -> p c a two", two=2),
                    in1=ex2A[:, :, None, :].to_broadcast([128, na, 64, 2]),
                    op=ALU.mult,
                )
                if nb_:
                    twB = spool.tile([128, nb_], F32, tag="twB")
                    nc.scalar.activation(
                        twB[:], esw_t[:, TOTA + b0 : TOTA + b1], AF.Tanh,
                        scale=0.5,
                    )
                    ex2B = spool.tile([128, nb_, 2], F16, tag="ex2B")
                    nc.scalar.activation(
                        ex2B[:],
                        twB[:, :, None].to_broadcast([128, nb_, 2]),
                        AF.Exp,
                        scale=0.5,
                    )
                    gsB = mpool.tile([128, nb_ * HID], F16, tag="gsB")
                    nc.vector.tensor_tensor(
                        out=gsB[:].rearrange(
                            "p (c a two) -> p c a two", a=64, two=2
                        ),
                        in0=gB3[:, :nb_].rearrange(
                            "p c (a two) -> p c a two", two=2
                        ),
                        in1=ex2B[:, :, None, :].to_broadcast([128, nb_, 64, 2]),
                        op=ALU.mult,
                    )
                for w in range(w0, w1_):
                    ncA = int(nchA[w])
                    ncB = int(nchB[w])
                    loA = int(offsA[w]) - a0
                    loB = int(offsB[w]) - b0
                    den = spool.tile([128, 1], F32, tag="den")
                    nc.vector.reduce_sum(
                        den[:], ex2A[:, loA : loA + ncA, 0],
                        axis=mybir.AxisListType.X,
                    )
                    if ncB:
                        denB = spool.tile([128, 1], F32, tag="denB")
                        nc.vector.reduce_sum(
                            denB[:], ex2B[:, loB : loB + ncB, 0],
                            axis=mybir.AxisListType.X,
                        )
                        nc.vector.tensor_tensor(
                            out=den[:], in0=den[:], in1=denB[:], op=ALU.add
                        )
                    nc.vector.tensor_scalar(
                        out=den[:], in0=den[:], scalar1=pcw_t[:, w : w + 1],
                        scalar2=0.5, op0=ALU.subtract, op1=ALU.max,
                    )
                    recip = spool.tile([128, 1], F32, tag="recip")
                    nc.vector.reciprocal(recip[:], den[:])
                    acc = psum.tile([128, HID], F32, tag="acc")
                    ncht = ncA + ncB
                    for c in range(ncA):
                        nc.tensor.matmul(
                            out=acc[:],
                            lhsT=ident[:],
                            rhs=gsA[:, (loA + c) * HID : (loA + c + 1) * HID],
                            start=(c == 0),
                            stop=(c == ncht - 1),
                        )
                    for c in range(ncB):
                        nc.tensor.matmul(
                            out=acc[:],
                            lhsT=ident[:],
                            rhs=gsB[:, (loB + c) * HID : (loB + c + 1) * HID],
                            start=False,
                            stop=(ncA + c == ncht - 1),
                        )
                    # ELU+1: max(x,0) + exp(min(x,0)), x = acc*recip
                    mm = spool.tile([128, HID], F32, tag="mm")
                    nc.vector.tensor_scalar(
                        out=mm[:], in0=acc[:], scalar1=recip[:],
                        scalar2=0.0, op0=ALU.mult, op1=ALU.min,
                    )
                    rr = spool.tile([128, HID], F32, tag="rr")
                    nc.vector.tensor_scalar(
                        out=rr[:], in0=acc[:], scalar1=recip[:],
                        scalar2=0.0, op0=ALU.mult, op1=ALU.max,
                    )
                    ee = spool.tile([128, HID], F32, tag="ee")
                    nc.scalar.activation(ee[:], mm[:], AF.Exp)
                    h1 = hpool.tile([128, HID], F16, tag="h1")
                    nc.vector.tensor_tensor(
                        out=h1[:], in0=rr[:], in1=ee[:], op=ALU.add
                    )
                    # yT_w = W2^T @ h1^T - colsum(W2) (the ELU -1 term)
                    h1tp = psumt.tile([128, HID], F16, tag="h1tp")
                    nc.tensor.transpose(
                        out=h1tp[:], in_=h1[:], identity=ident[:]
                    )
                    h1t = hpool.tile([128, HID], F16, tag="h1t")
                    nc.scalar.activation(h1t[:], h1tp[:], AF.Copy)
                    yp = psumy.tile([OUT, 128], F32, tag="yp")
                    nc.tensor.matmul(
                        out=yp[:], lhsT=w2_t[:], rhs=h1t[:],
                        start=True, stop=True,
                    )
                    yt = hpool.tile([OUT, 128], F16, tag="yt")
                    nc.scalar.activation(
                        yt[:], yp[:], AF.Identity, bias=c2n_t[:]
                    )
                    nc.sync.dma_start(
                        out=yT[:, w * 128 : (w + 1) * 128], in_=yt[:]
                    )
    nc.finalize()
    return nc


# ---------------------------------------------------------------- host glue
def _wrap16(idx_cols):
    """[128, ncols] int16 slot indices -> dma_gather wrapped layout.

    Flattened order i = c*128 + p; idx i lives at (partition i%16,
    col i//16), replicated across the 8 groups of 16 partitions.
    Returns [128, 8*ncols]."""
    ncols = idx_cols.shape[1]
    flat = idx_cols.T.reshape(-1)                 # i = c*128 + p
    wrapped = flat.reshape(8 * ncols, 16).T       # [16, 8*ncols]
    return np.tile(wrapped, (8, 1)).astype(np.int16)


def kernel(x, edge_index, W1, att_src, att_dst, W2):
    x = np.asarray(x, dtype=np.float32)
    edge_index = np.asarray(edge_index)
    W1 = np.asarray(W1, dtype=np.float32)
    att_src = np.asarray(att_src, dtype=np.float32)
    att_dst = np.asarray(att_dst, dtype=np.float32)
    W2 = np.asarray(W2, dtype=np.float32)

    src = edge_index[0].astype(np.int64)
    dst = edge_index[1].astype(np.int64)

    # ---- phase 1: sharded h/es/ed compute
    wcat = np.concatenate(
        [W1, (W1 @ att_src)[:, None], (W1 @ att_dst)[:, None]], axis=1
    ).astype(np.float16)
    xT = np.ascontiguousarray(x.T).astype(np.float16)  # [IN, N]

    nc1 = build_phase1()
    in_maps1 = []
    for c in range(NCORES):
        sh = xT[:, c * NPC : (c + 1) * NPC]
        if sh.shape[1] < NPAD:
            sh = np.concatenate(
                [sh, np.zeros((IN, NPAD - sh.shape[1]), np.float16)], axis=1
            )
        in_maps1.append({"xT": np.ascontiguousarray(sh), "wcat": wcat})
    trace = os.environ.get("BASS_GAT_TRACE") == "1"
    tkw = dict(trace=True, trace_cores=[0]) if trace else {}
    if trace:
        _patch_perfetto()
    t0 = time.time()
    res1 = run_bass_kernel_spmd(nc1, in_maps1, core_ids=list(range(NCORES)), **tkw)
    _timings["phase1_wall"] = time.time() - t0
    _timings["phase1_ns"] = res1.exec_time_ns

    htab = np.zeros((TABROWS, HID), np.float16)
    es_full = np.zeros(N, np.float32)
    ed_full = np.zeros(N, np.float32)
    for c in range(NCORES):
        hv = res1.results[c]["haug"][:NPC]
        htab[1 + c * NPC : 1 + (c + 1) * NPC] = hv[:, :HID]
        es_full[c * NPC : (c + 1) * NPC] = hv[:, HID].astype(np.float32)
        ed_full[c * NPC : (c + 1) * NPC] = hv[:, HID + 1].astype(np.float32)

    # ---- host edge routing
    deg = np.bincount(dst, minlength=N)
    is_low = (src + 1) < LOWROWS
    degA_full = np.bincount(dst[is_low], minlength=N)
    degB_full = deg - degA_full

    # per-node low/high src lists (dst-sorted edge order)
    lkey = dst * 2 + (~is_low).astype(np.int64)   # low edges first per dst
    eorder = np.argsort(lkey, kind="stable")
    src_s = src[eorder]
    estarts = np.zeros(N + 1, np.int64)
    estarts[1:] = np.cumsum(deg)

    orders = []
    nchA_pc = np.zeros((NCORES, NW), np.int64)
    nchB_pc = np.zeros((NCORES, NW), np.int64)
    for c in range(NCORES):
        sl = slice(c * NPC, (c + 1) * NPC)
        dA, dB = degA_full[sl], degB_full[sl]
        order = np.lexsort((-dB, -dA))
        orders.append(order)
        dAs, dBs = dA[order], dB[order]
        for w in range(NW):
            j0 = w * 128
            if j0 < NPC:
                j1 = min(j0 + 128, NPC)
                nchA_pc[c, w] = dAs[j0:j1].max()
                nchB_pc[c, w] = dBs[j0:j1].max()
    nchA = np.maximum(nchA_pc.max(axis=0), 1)
    nchB = nchB_pc.max(axis=0)
    offsA = np.zeros(NW + 1, np.int64)
    offsA[1:] = np.cumsum(nchA)
    offsB = np.zeros(NW + 1, np.int64)
    offsB[1:] = np.cumsum(nchB)
    TOTA, TOTB = int(offsA[-1]), int(offsB[-1])

    groups = []
    w0 = 0
    while w0 < NW:
        w1_ = w0 + 1
        while w1_ < NW and (
            (offsA[w1_ + 1] - offsA[w0]) + (offsB[w1_ + 1] - offsB[w0])
            <= GCOLS
        ):
            w1_ += 1
        groups.append((w0, w1_))
        w0 = w1_

    in_maps2 = []
    for c in range(NCORES):
        order = orders[c]
        idxA = np.zeros((128, TOTA), np.int64)            # dummy low row 0
        idxB = np.full((128, TOTB), DUMMY_HI_LOCAL, np.int64)
        esw_arr = np.full((128, TOTA + TOTB), -60000.0, np.float32)
        padcnt = np.zeros((128, NW), np.float32)
        for wloc in range(NW):
            j0 = wloc * 128
            nodes = order[j0 : j0 + 128]
            ncA, ncB = int(nchA[wloc]), int(nchB[wloc])
            for p, j in enumerate(nodes):
                g = c * NPC + j
                dA = int(degA_full[g])
                dB = int(degB_full[g])
                s0 = estarts[g]
                ssA = src_s[s0 : s0 + dA]               # low edges first
                ssB = src_s[s0 + dA : s0 + dA + dB]
                colA = int(offsA[wloc])
                colB = int(offsB[wloc])
                idxA[p, colA : colA + dA] = ssA + 1
                idxB[p, colB : colB + dB] = ssB + 1 - HIBASE
                esw_arr[p, colA : colA + dA] = es_full[ssA] + ed_full[g]
                esw_arr[p, colA + dA : colA + ncA] += ed_full[g]
                esw_arr[p, TOTA + colB : TOTA + colB + dB] = (
                    es_full[ssB] + ed_full[g]
                )
                esw_arr[p, TOTA + colB + dB : TOTA + colB + ncB] += ed_full[g]
                padcnt[p, wloc] = (ncA - dA) + (ncB - dB)
            for p in range(len(nodes), 128):
                padcnt[p, wloc] = ncA + ncB
        iparts = []
        for (gw0, gw1) in groups:
            iparts.append(
                _wrap16(idxA[:, offsA[gw0] : offsA[gw1]].astype(np.int16))
            )
            if offsB[gw1] > offsB[gw0]:
                iparts.append(
                    _wrap16(idxB[:, offsB[gw0] : offsB[gw1]].astype(np.int16))
                )
        idxs_full = np.ascontiguousarray(np.concatenate(iparts, axis=1))
        in_maps2.append(
            {
                "htab": htab,
                "idxs": idxs_full,
                "esw": esw_arr.astype(np.float16),
                "pcw": (padcnt * np.float32(np.exp(-0.5))).astype(np.float32),
                "w2": W2.astype(np.float16),
                "c2n": -W2.sum(axis=0, dtype=np.float32)[:, None],
            }
        )

    nc2 = build_phase2(nchA, nchB, groups)
    t0 = time.time()
    res2 = run_bass_kernel_spmd(nc2, in_maps2, core_ids=list(range(NCORES)), **tkw)
    _timings["phase2_wall"] = time.time() - t0
    _timings["phase2_ns"] = res2.exec_time_ns

    out = np.zeros((N, OUT), np.float32)
    for c in range(NCORES):
        yv = res2.results[c]["yT"].astype(np.float32).T  # [NPAD, OUT]
        order = orders[c]
        out[c * NPC + order] = yv[:NPC]
    return out


# revision 8
# speedup vs baseline: 2.2617x; 1.9286x over previous
"""GAT encoder (gnn_message_passing) on 8 trn2 NeuronCores via Bass.

Strategy (graph-parallel, dst-sharded), v2:
  Phase 1 (node-sharded): one fp16 matmul chain per 128-node tile against
    [W1 | W1@att_src | W1@att_dst] -> rows [h(128) | es | ed] fp16.
  Host: build gather table htab (h rows, 256B fp16), route edges to
    dst-owner cores into windows of 128 dst nodes sorted by (low-degree,
    high-degree); per-slot es+ed bias; per-window pad counts.
  Phase 2 (per core): dma_gather bulk-fetches edge-slot h rows.  int16
    indices limit a gather table to 32768 rows, so each window has
    nchA low-src columns + nchB high-src columns, fetched by two calls
    per group from the two table halves (pad slots read a zero dummy row;
    every index is valid - HW emits descriptors for the first
    num_idxs_reg entries and skips nothing).  Attention without act-table
    swaps: exp(sigmoid(z)) = e^0.5 * exp(tanh(z/2)/2); the constant
    cancels in the softmax (pad slots use es=-60000 -> e^-0.5, removed
    from the denominator via pcw).  ex applied via one DVE fp16 multiply
    per pass per group; per-window PSUM accumulation via fp16 identity
    matmuls; ELU via max(x,0)+exp(min(x,0))-1 with the -1 folded into
    the output bias; y produced transposed and fixed up on host.
"""
import os
import sys
import time

sys.path.insert(0, "/opt/trn_rl_repo")

import numpy as np

N, E = 50000, 800000
IN, HID, OUT = 256, 128, 128
NCORES = 8
NPC = N // NCORES            # nodes per core (6250)
NW = (NPC + 127) // 128      # 49 windows; last partial (6250 = 48*128+106)
NPAD = NW * 128              # 6272
GCOLS = 64                   # max total gather columns per group
CALLCOLS = int(os.environ.get("BASS_GAT_CALLCOLS", "64"))
SINGLE_PACKET = os.environ.get("BASS_GAT_SP", "0") == "1"

TABROWS = N + 3              # row 0 = zero (low dummy), 1..N = node src+1,
LOWROWS = 32768              # row N+2 = zero (high dummy)
HIBASE = 32767
DUMMY_HI_LOCAL = N + 2 - HIBASE

_timings = {}


def _patch_env():
    """Tile/perfetto compatibility patches for this container."""
    import concourse.tile as tile
    from concourse.tile import ScopedClock

    def _drain_and_barrier_split(self, tick_clock, wait_clock):
        nc = self.nc
        probe = nc.sync.nop()
        wait_clock.add_sem_waits(
            probe.ins, ScopedClock({None: tick_clock.global_clock})
        )
        waits = list(probe.ins.sync_info.on_wait or [])
        probe.ins.sync_info.on_wait = []
        from concourse import mybir

        for w in waits:
            inst = nc.sync.nop()
            if inst.ins.sync_info is None:
                inst.ins.sync_info = mybir.SyncInfo(on_wait=[w], on_update=[])
            else:
                inst.ins.sync_info.on_wait = [w]
        nc.sync.drain()
        nc.all_engine_barrier()
        assert self.sems is not None
        popped = nc._tile_sem_poison_stack.pop()
        assert popped is self._sem_poison
        nc.clear_and_free_semaphores(list(self.sems.allocated().values()))
        nc.all_engine_barrier()

    tile.TileContext._drain_and_barrier = _drain_and_barrier_split


_patch_env()


def _patch_perfetto():
    try:
        from gauge import trn_perfetto

        cls = trn_perfetto.TrnPerfettoConv
        if not getattr(cls, "_no_hlo_patched", False):
            _orig_init = cls.__init__

            def _init_no_hlo(self, *a, **k):
                k["annotate_hlo"] = False
                if len(a) >= 2:
                    a = (a[0], False) + a[2:]
                _orig_init(self, *a, **k)

            cls.__init__ = _init_no_hlo
            cls._no_hlo_patched = True
    except Exception:
        pass


import concourse.bass as bass
import concourse.bacc as bacc
import concourse.tile as tile
from concourse import mybir
from concourse.bass_utils import run_bass_kernel_spmd
from concourse.masks import make_identity

F32 = mybir.dt.float32
F16 = mybir.dt.float16
I16 = mybir.dt.int16
AF = mybir.ActivationFunctionType
ALU = mybir.AluOpType


# ---------------------------------------------------------------- phase 1
def build_phase1():
    """h/es/ed for this core's nodes: one fp16 matmul chain per tile."""
    nc = bacc.Bacc("TRN2", target_bir_lowering=True)
    xT = nc.dram_tensor("xT", [IN, NPAD], F16, kind="ExternalInput")
    wcat = nc.dram_tensor("wcat", [IN, HID + 2], F16, kind="ExternalInput")
    haug = nc.dram_tensor("haug", [NPAD, HID + 2], F16, kind="ExternalOutput")

    RF = HID + 2  # 130
    TB = 4        # tiles per DMA batch

    with tile.TileContext(nc) as tc:
        with (
            tc.tile_pool(name="sbuf", bufs=3) as pool,
            tc.tile_pool(name="cpool", bufs=1) as cpool,
            tc.tile_pool(name="psum", bufs=4, space="PSUM") as psum,
        ):
            w_t = cpool.tile([128, IN // 128, RF], F16)
            nc.sync.dma_start(
                out=w_t[:], in_=wcat[:].rearrange("(a k) f -> k a f", k=128)
            )
            nb = (NW + TB - 1) // TB
            for b in range(nb):
                t0 = b * TB
                tn = min(TB, NW - t0)
                xt = pool.tile([128, IN // 128, TB * 128], F16, tag="xt")
                nc.sync.dma_start(
                    out=xt[:, :, : tn * 128],
                    in_=xT[:, t0 * 128 : (t0 + tn) * 128].rearrange(
                        "(a k) n -> k a n", k=128
                    ),
                )
                ha = pool.tile([128, TB, RF], F16, tag="ha")
                for t in range(tn):
                    hp = psum.tile([128, RF], F32, tag="hp")
                    for a in range(IN // 128):
                        nc.tensor.matmul(
                            out=hp[:],
                            lhsT=xt[:, a, t * 128 : (t + 1) * 128],
                            rhs=w_t[:, a],
                            start=(a == 0),
                            stop=(a == IN // 128 - 1),
                        )
                    nc.scalar.activation(ha[:, t], hp[:], AF.Copy)
                nc.sync.dma_start(
                    out=haug[t0 * 128 : (t0 + tn) * 128, :].rearrange(
                        "(t k) f -> k t f", k=128
                    ),
                    in_=ha[:, :tn],
                )
    nc.finalize()
    return nc


# ---------------------------------------------------------------- phase 2
def build_phase2(nchA, nchB, groups):
    """Per-window low/high chunk counts and group spans."""
    NWl = len(nchA)
    offsA = np.zeros(NWl + 1, dtype=int)
    offsA[1:] = np.cumsum(nchA)
    offsB = np.zeros(NWl + 1, dtype=int)
    offsB[1:] = np.cumsum(nchB)
    TOTA, TOTB = int(offsA[-1]), int(offsB[-1])
    # idx segment start per group (8 wrapped cols per gather col, A then B)
    ico = np.zeros(len(groups) + 1, dtype=int)
    for g, (w0, w1_) in enumerate(groups):
        na = int(offsA[w1_] - offsA[w0])
        nb_ = int(offsB[w1_] - offsB[w0])
        ico[g + 1] = ico[g] + 8 * (na + nb_)
    ITOT = int(ico[-1])

    nc = bacc.Bacc("TRN2", target_bir_lowering=True, num_swdge_queues=4)
    htab = nc.dram_tensor("htab", [TABROWS, HID], F16, kind="ExternalInput")
    idxs = nc.dram_tensor("idxs", [128, ITOT], I16, kind="ExternalInput")
    esw = nc.dram_tensor("esw", [128, TOTA + TOTB], F16, kind="ExternalInput")
    pcw = nc.dram_tensor("pcw", [128, NW], F32, kind="ExternalInput")
    w2 = nc.dram_tensor("w2", [HID, OUT], F16, kind="ExternalInput")
    c2n = nc.dram_tensor("c2n", [OUT, 1], F32, kind="ExternalInput")
    yT = nc.dram_tensor("yT", [OUT, NW * 128], F16, kind="ExternalOutput")

    with tile.TileContext(nc) as tc:
        with (
            tc.tile_pool(name="gpool", bufs=3) as gpool,
            tc.tile_pool(name="mpool", bufs=2) as mpool,
            tc.tile_pool(name="spool", bufs=6) as spool,
            tc.tile_pool(name="hpool", bufs=3) as hpool,
            tc.tile_pool(name="cpool", bufs=1) as cpool,
            tc.tile_pool(name="psum", bufs=4, space="PSUM") as psum,
            tc.tile_pool(name="psumt", bufs=2, space="PSUM") as psumt,
            tc.tile_pool(name="psumy", bufs=2, space="PSUM") as psumy,
        ):
            ident = cpool.tile([128, 128], F16)
            make_identity(nc, ident[:])
            w2_t = cpool.tile([HID, OUT], F16)
            nc.sync.dma_start(out=w2_t[:], in_=w2[:])
            c2n_t = cpool.tile([OUT, 1], F32)
            nc.sync.dma_start(out=c2n_t[:], in_=c2n[:])
            pcw_t = cpool.tile([128, NW], F32)
            nc.sync.dma_start(out=pcw_t[:], in_=pcw[:])
            it_t = cpool.tile([128, ITOT], I16)
            nc.sync.dma_start(out=it_t[:], in_=idxs[:])
            qrr = [0]
            esw_t = cpool.tile([128, TOTA + TOTB], F16)
            nc.sync.dma_start(out=esw_t[:], in_=esw[:])

            for g, (w0, w1_) in enumerate(groups):
                a0, a1 = int(offsA[w0]), int(offsA[w1_])
                b0, b1 = int(offsB[w0]), int(offsB[w1_])
                na, nb_ = a1 - a0, b1 - b0
                i0 = int(ico[g])

                gtA = gpool.tile([128, na * HID], F16, tag="gtA")
                gA3 = gtA[:].rearrange("p (c f) -> p c f", f=HID)
                for s0 in range(0, na, CALLCOLS):
                    sn = min(CALLCOLS, na - s0)
                    nc.gpsimd.dma_gather(
                        out_ap=gA3[:, s0 : s0 + sn],
                        in_ap=htab[:LOWROWS],
                        idxs_ap=it_t[:, i0 + 8 * s0 : i0 + 8 * (s0 + sn)],
                        num_idxs=128 * sn,
                        num_idxs_reg=128 * sn,
                        elem_size=HID,
                        single_packet=SINGLE_PACKET,
                        queue_num=qrr[0] % 4,
                    )
                    qrr[0] += 1
                gtB = gpool.tile([128, max(nb_, 1) * HID], F16, tag="gtB")
                gB3 = gtB[:].rearrange("p (c f) -> p c f", f=HID)
                for s0 in range(0, nb_, CALLCOLS):
                    sn = min(CALLCOLS, nb_ - s0)
                    nc.gpsimd.dma_gather(
                        out_ap=gB3[:, s0 : s0 + sn],
                        in_ap=htab[HIBASE:],
                        idxs_ap=it_t[
                            :, i0 + 8 * (na + s0) : i0 + 8 * (na + s0 + sn)
                        ],
                        num_idxs=128 * sn,
                        num_idxs_reg=128 * sn,
                        elem_size=HID,
                        single_packet=SINGLE_PACKET,
                        queue_num=qrr[0] % 4,
                    )
                    qrr[0] += 1
                # t = tanh(z/2); ex = exp(t/2)  (softmax scale e^0.5 cancels)
                twA = spool.tile([128, na], F32, tag="twA")
                nc.scalar.activation(
                    twA[:], esw_t[:, a0:a1], AF.Tanh, scale=0.5
                )
                ex2A = spool.tile([128, na, 2], F16, tag="ex2A")
                nc.scalar.activation(
                    ex2A[:],
                    twA[:, :, None].to_broadcast([128, na, 2]),
                    AF.Exp,
                    scale=0.5,
                )
                gsA = mpool.tile([128, na * HID], F16, tag="gsA")
                nc.vector.tensor_tensor(
                    out=gsA[:].rearrange("p (c a two) -> p c a two", a=64, two=2),
                    in0=gA3.rearrange("p c (a two) -> p c a two", two=2),
                    in1=ex2A[:, :, None, :].to_broadcast([128, na, 64, 2]),
                    op=ALU.mult,
                )
                if nb_:
                    twB = spool.tile([128, nb_], F32, tag="twB")
                    nc.scalar.activation(
                        twB[:], esw_t[:, TOTA + b0 : TOTA + b1], AF.Tanh,
                        scale=0.5,
                    )
                    ex2B = spool.tile([128, nb_, 2], F16, tag="ex2B")
                    nc.scalar.activation(
                        ex2B[:],
                        twB[:, :, None].to_broadcast([128, nb_, 2]),
                        AF.Exp,
                        scale=0.5,
                    )
                    gsB = mpool.tile([128, nb_ * HID], F16, tag="gsB")
                    nc.vector.tensor_tensor(
                        out=gsB[:].rearrange(
                            "p (c a two) -> p c a two", a=64, two=2
                        ),
                        in0=gB3[:, :nb_].rearrange(
                            "p c (a two) -> p c a two", two=2
                        ),
                        in1=ex2B[:, :, None, :].to_broadcast([128, nb_, 64, 2]),
                        op=ALU.mult,
                    )
                for w in range(w0, w1_):
                    ncA = int(nchA[w])
                    ncB = int(nchB[w])
                    loA = int(offsA[w]) - a0
                    loB = int(offsB[w]) - b0
                    den = spool.tile([128, 1], F32, tag="den")
                    nc.vector.reduce_sum(
                        den[:], ex2A[:, loA : loA + ncA, 0],
                        axis=mybir.AxisListType.X,
                    )
                    if ncB:
                        denB = spool.tile([128, 1], F32, tag="denB")
                        nc.vector.reduce_sum(
                            denB[:], ex2B[:, loB : loB + ncB, 0],
                            axis=mybir.AxisListType.X,
                        )
                        nc.vector.tensor_tensor(
                            out=den[:], in0=den[:], in1=denB[:], op=ALU.add
                        )
                    nc.vector.tensor_scalar(
                        out=den[:], in0=den[:], scalar1=pcw_t[:, w : w + 1],
                        scalar2=0.5, op0=ALU.subtract, op1=ALU.max,
                    )
                    recip = spool.tile([128, 1], F32, tag="recip")
                    nc.vector.reciprocal(recip[:], den[:])
                    acc = psum.tile([128, HID], F32, tag="acc")
                    ncht = ncA + ncB
                    for c in range(ncA):
                        nc.tensor.matmul(
                            out=acc[:],
                            lhsT=ident[:],
                            rhs=gsA[:, (loA + c) * HID : (loA + c + 1) * HID],
                            start=(c == 0),
                            stop=(c == ncht - 1),
                        )
                    for c in range(ncB):
                        nc.tensor.matmul(
                            out=acc[:],
                            lhsT=ident[:],
                            rhs=gsB[:, (loB + c) * HID : (loB + c + 1) * HID],
                            start=False,
                            stop=(ncA + c == ncht - 1),
                        )
                    # ELU+1: max(x,0) + exp(min(x,0)), x = acc*recip
                    mm = spool.tile([128, HID], F32, tag="mm")
                    nc.vector.tensor_scalar(
                        out=mm[:], in0=acc[:], scalar1=recip[:],
                        scalar2=0.0, op0=ALU.mult, op1=ALU.min,
                    )
                    rr = spool.tile([128, HID], F32, tag="rr")
                    nc.vector.tensor_scalar(
                        out=rr[:], in0=acc[:], scalar1=recip[:],
                        scalar2=0.0, op0=ALU.mult, op1=ALU.max,
                    )
                    ee = spool.tile([128, HID], F32, tag="ee")
                    nc.scalar.activation(ee[:], mm[:], AF.Exp)
                    h1 = hpool.tile([128, HID], F16, tag="h1")
                    nc.vector.tensor_tensor(
                        out=h1[:], in0=rr[:], in1=ee[:], op=ALU.add
                    )
                    # yT_w = W2^T @ h1^T - colsum(W2) (the ELU -1 term)
                    h1tp = psumt.tile([128, HID], F16, tag="h1tp")
                    nc.tensor.transpose(
                        out=h1tp[:], in_=h1[:], identity=ident[:]
                    )
                    h1t = hpool.tile([128, HID], F16, tag="h1t")
                    nc.scalar.activation(h1t[:], h1tp[:], AF.Copy)
                    yp = psumy.tile([OUT, 128], F32, tag="yp")
                    nc.tensor.matmul(
                        out=yp[:], lhsT=w2_t[:], rhs=h1t[:],
                        start=True, stop=True,
                    )
                    yt = hpool.tile([OUT, 128], F16, tag="yt")
                    nc.scalar.activation(
                        yt[:], yp[:], AF.Identity, bias=c2n_t[:]
                    )
                    nc.sync.dma_start(
                        out=yT[:, w * 128 : (w + 1) * 128], in_=yt[:]
                    )
    nc.finalize()
    return nc


# ---------------------------------------------------------------- host glue
def _wrap16(idx_cols):
    """[128, ncols] int16 slot indices -> dma_gather wrapped layout.

    Flattened order i = c*128 + p; idx i lives at (partition i%16,
    col i//16), replicated across the 8 groups of 16 partitions.
    Returns [128, 8*ncols]."""
    ncols = idx_cols.shape[1]
    flat = idx_cols.T.reshape(-1)                 # i = c*128 + p
    wrapped = flat.reshape(8 * ncols, 16).T       # [16, 8*ncols]
    return np.tile(wrapped, (8, 1)).astype(np.int16)


def kernel(x, edge_index, W1, att_src, att_dst, W2):
    x = np.asarray(x, dtype=np.float32)
    edge_index = np.asarray(edge_index)
    W1 = np.asarray(W1, dtype=np.float32)
    att_src = np.asarray(att_src, dtype=np.float32)
    att_dst = np.asarray(att_dst, dtype=np.float32)
    W2 = np.asarray(W2, dtype=np.float32)

    src = edge_index[0].astype(np.int64)
    dst = edge_index[1].astype(np.int64)

    # ---- phase 1: sharded h/es/ed compute
    wcat = np.concatenate(
        [W1, (W1 @ att_src)[:, None], (W1 @ att_dst)[:, None]], axis=1
    ).astype(np.float16)
    xT = np.ascontiguousarray(x.T).astype(np.float16)  # [IN, N]

    nc1 = build_phase1()
    in_maps1 = []
    for c in range(NCORES):
        sh = xT[:, c * NPC : (c + 1) * NPC]
        if sh.shape[1] < NPAD:
            sh = np.concatenate(
                [sh, np.zeros((IN, NPAD - sh.shape[1]), np.float16)], axis=1
            )
        in_maps1.append({"xT": np.ascontiguousarray(sh), "wcat": wcat})
    trace = os.environ.get("BASS_GAT_TRACE") == "1"
    tkw = dict(trace=True, trace_cores=[0]) if trace else {}
    if trace:
        _patch_perfetto()
    t0 = time.time()
    res1 = run_bass_kernel_spmd(nc1, in_maps1, core_ids=list(range(NCORES)), **tkw)
    _timings["phase1_wall"] = time.time() - t0
    _timings["phase1_ns"] = res1.exec_time_ns

    htab = np.zeros((TABROWS, HID), np.float16)
    es_full = np.zeros(N, np.float32)
    ed_full = np.zeros(N, np.float32)
    for c in range(NCORES):
        hv = res1.results[c]["haug"][:NPC]
        htab[1 + c * NPC : 1 + (c + 1) * NPC] = hv[:, :HID]
        es_full[c * NPC : (c + 1) * NPC] = hv[:, HID].astype(np.float32)
        ed_full[c * NPC : (c + 1) * NPC] = hv[:, HID + 1].astype(np.float32)

    # ---- host edge routing
    deg = np.bincount(dst, minlength=N)
    is_low = (src + 1) < LOWROWS
    degA_full = np.bincount(dst[is_low], minlength=N)
    degB_full = deg - degA_full

    # per-node low/high src lists (dst-sorted edge order)
    lkey = dst * 2 + (~is_low).astype(np.int64)   # low edges first per dst
    eorder = np.argsort(lkey, kind="stable")
    src_s = src[eorder]
    estarts = np.zeros(N + 1, np.int64)
    estarts[1:] = np.cumsum(deg)

    orders = []
    nchA_pc = np.zeros((NCORES, NW), np.int64)
    nchB_pc = np.zeros((NCORES, NW), np.int64)
    for c in range(NCORES):
        sl = slice(c * NPC, (c + 1) * NPC)
        dA, dB = degA_full[sl], degB_full[sl]
        order = np.lexsort((-dB, -dA))
        orders.append(order)
        dAs, dBs = dA[order], dB[order]
        for w in range(NW):
            j0 = w * 128
            if j0 < NPC:
                j1 = min(j0 + 128, NPC)
                nchA_pc[c, w] = dAs[j0:j1].max()
                nchB_pc[c, w] = dBs[j0:j1].max()
    nchA = np.maximum(nchA_pc.max(axis=0), 1)
    nchB = nchB_pc.max(axis=0)
    offsA = np.zeros(NW + 1, np.int64)
    offsA[1:] = np.cumsum(nchA)
    offsB = np.zeros(NW + 1, np.int64)
    offsB[1:] = np.cumsum(nchB)
    TOTA, TOTB = int(offsA[-1]), int(offsB[-1])

    groups = []
    w0 = 0
    while w0 < NW:
        w1_ = w0 + 1
        while w1_ < NW and (
            (offsA[w1_ + 1] - offsA[w0]) + (offsB[w1_ + 1] - offsB[w0])
            <= GCOLS
        ):
            w1_ += 1
        groups.append((w0, w1_))
        w0 = w1_

    in_maps2 = []
    for c in range(NCORES):
        order = orders[c]
        idxA = np.zeros((128, TOTA), np.int64)            # dummy low row 0
        idxB = np.full((128, TOTB), DUMMY_HI_LOCAL, np.int64)
        esw_arr = np.full((128, TOTA + TOTB), -60000.0, np.float32)
        padcnt = np.zeros((128, NW), np.float32)
        for wloc in range(NW):
            j0 = wloc * 128
            nodes = order[j0 : j0 + 128]
            ncA, ncB = int(nchA[wloc]), int(nchB[wloc])
            for p, j in enumerate(nodes):
                g = c * NPC + j
                dA = int(degA_full[g])
                dB = int(degB_full[g])
                s0 = estarts[g]
                ssA = src_s[s0 : s0 + dA]               # low edges first
                ssB = src_s[s0 + dA : s0 + dA + dB]
                colA = int(offsA[wloc])
                colB = int(offsB[wloc])
                idxA[p, colA : colA + dA] = ssA + 1
                idxB[p, colB : colB + dB] = ssB + 1 - HIBASE
                esw_arr[p, colA : colA + dA] = es_full[ssA] + ed_full[g]
                esw_arr[p, colA + dA : colA + ncA] += ed_full[g]
                esw_arr[p, TOTA + colB : TOTA + colB + dB] = (
                    es_full[ssB] + ed_full[g]
                )
                esw_arr[p, TOTA + colB + dB : TOTA + colB + ncB] += ed_full[g]
                padcnt[p, wloc] = (ncA - dA) + (ncB - dB)
            for p in range(len(nodes), 128):
                padcnt[p, wloc] = ncA + ncB
        iparts = []
        for (gw0, gw1) in groups:
            iparts.append(
                _wrap16(idxA[:, offsA[gw0] : offsA[gw1]].astype(np.int16))
            )
            if offsB[gw1] > offsB[gw0]:
                iparts.append(
                    _wrap16(idxB[:, offsB[gw0] : offsB[gw1]].astype(np.int16))
                )
        idxs_full = np.ascontiguousarray(np.concatenate(iparts, axis=1))
        in_maps2.append(
            {
                "htab": htab,
                "idxs": idxs_full,
                "esw": esw_arr.astype(np.float16),
                "pcw": (padcnt * np.float32(np.exp(-0.5))).astype(np.float32),
                "w2": W2.astype(np.float16),
                "c2n": -W2.sum(axis=0, dtype=np.float32)[:, None],
            }
        )

    nc2 = build_phase2(nchA, nchB, groups)
    t0 = time.time()
    res2 = run_bass_kernel_spmd(nc2, in_maps2, core_ids=list(range(NCORES)), **tkw)
    _timings["phase2_wall"] = time.time() - t0
    _timings["phase2_ns"] = res2.exec_time_ns

    out = np.zeros((N, OUT), np.float32)
    for c in range(NCORES):
        yv = res2.results[c]["yT"].astype(np.float32).T  # [NPAD, OUT]
        order = orders[c]
        out[c * NPC + order] = yv[:NPC]
    return out
